# revision 10
# baseline (speedup 1.0000x reference)
"""Trainium2 Bass kernel for nn_EquivariantDense.

Reference computation (per sample b of 64):
    rots  = stack([rot90(w_b, k, axes=(0,1)) for k in range(4)], axis=3)   # (8,8,64,4,15)
    filt  = rots.reshape(16384, 15).T                                      # (15, 16384)
    out_b = filt @ x_b                                                     # (15,)

Key algebraic reduction (4x less compute, no filter expansion):
    out_b[o] = sum_{s,c} w_b[s,c,o] * y_b[s,c]
    y_b      = sum_k rot90(x_b[..., k], -k)          (x_b viewed as (8,8,64,4))

Sharding: data-parallel over the batch-of-64 -> 8 samples per NeuronCore.
Per core the device kernel does:
    - DMA  XR (128, 1024) f32: the 4 rotation-gathered copies of x, laid out
      [p=(s'%2)*64+c, f=(k,t,b)] with t = s'//2
    - DMA  WT (128, 3840) f32: w laid out [p=j'%128, f=(t,b,o)], j' = t*128+p
    - VE:  y = sum of the 4 k-slices                         -> (128, 256)
    - PE:  32 accumulating matmuls ps[8,120] += y_t.T @ w_t  (M=samples,
           N=(sample,out) pairs; the useful result is the diagonal blocks)
    - VE:  extract ps[b, b*15:(b+1)*15] -> out (8, 15), DMA out.

All layout permutation is done host-side (pure bijective reordering; full
input data is still shipped to the devices).
"""

import os
import sys
import types

import numpy as np


def _ensure_axon_ntff_hook():
    """The agent image's ``antenv`` lacks ``axon_hooks``; concourse's
    trace-under-axon path hard-imports it. Shim the module and register the
    real hook from trn_agent_boot so NTFF profiling works. Best-effort."""
    try:
        import antenv.axon_hooks  # noqa: F401
        return
    except ImportError:
        pass
    try:
        import antenv

        mod = types.ModuleType("antenv.axon_hooks")
        _hook = [None]
        mod.set_axon_ntff_profile_hook = lambda h: _hook.__setitem__(0, h)
        mod.get_axon_ntff_profile_hook = lambda: _hook[0]
        sys.modules["antenv.axon_hooks"] = mod
        antenv.axon_hooks = mod
        try:
            from trn_agent_boot.trn_boot import _ntff_profile_via_ctypes

            mod.set_axon_ntff_profile_hook(
                _ntff_profile_via_ctypes("/opt/axon/libaxon_pjrt.so")
            )
        except Exception:
            pass  # hook stays None -> concourse skips tracing gracefully
    except Exception:
        pass


_ensure_axon_ntff_hook()

B, H, Wd, C, K, OUT = 64, 8, 8, 64, 4, 15
NCORES = 8
BL = B // NCORES  # samples per core
T = 32            # K-chunks of 128 along the 4096 contraction
S = H * Wd        # 64 spatial positions

# "fp32" or "bf16" (bf16 halves both DMA bytes and PE time; fp32 is exact)
DTYPE = os.environ.get("EQ_KERNEL_DTYPE", "bf16")

_CACHE: dict = {}


def _build_nc(dtype_name: str):
    import concourse.mybir as mybir
    import concourse.tile as tile
    from concourse import bacc

    dt_in = mybir.dt.bfloat16 if dtype_name == "bf16" else mybir.dt.float32

    nc = bacc.Bacc(
        "TRN2",
        target_bir_lowering=False,
        debug=False,
        enable_asserts=False,
        num_devices=NCORES,
    )
    xr = nc.dram_tensor("xr", (128, K * T * BL), dt_in, kind="ExternalInput").ap()
    wt = nc.dram_tensor("wt", (128, T * BL * OUT), dt_in, kind="ExternalInput").ap()
    # (8, 120) staging: row bl holds all (sample, out) pairs; host slices the
    # diagonal blocks [bl, bl*15:(bl+1)*15] after gathering
    out = nc.dram_tensor(
        "out", (BL, BL * OUT), mybir.dt.float32, kind="ExternalOutput"
    ).ap()

    NW = 4                      # W is DMA'd in NW chunks to overlap with PE
    WCH = T * BL * OUT // NW    # 960 f32 per partition per chunk
    TCH = T // NW               # t-chunks covered per DMA chunk

    with tile.TileContext(nc) as tc:
        with (
            tc.tile_pool(name="io", bufs=1) as pool,
            tc.tile_pool(name="ps", bufs=1, space="PSUM") as psum_pool,
        ):
            # PE warmup: dummy matmuls on a zeroed tile keep the PE busy during
            # the input-DMA wait so HAM un-throttles (1.2 -> 2.4 GHz) before the
            # real matmuls run. No deps -> Tile schedules them first.
            warm = pool.tile([128, 512], dt_in, tag="warm")
            nc.vector.memset(warm[:], 0.0)
            ps_warm = psum_pool.tile([8, 512], mybir.dt.float32, tag="psw")
            for _ in range(8):
                nc.tensor.matmul(ps_warm[:], warm[:, :8], warm[:, :], start=True, stop=True)

            # Spread DMA issues across all three descriptor-gen paths so every
            # transfer is streaming by ~1.5us after the prologue: sync gets xr
            # (gates y -> all matmuls) then wt1; scalar gets wt0/wt3; the
            # SWDGE (gpsimd) gets wt2.
            xr_t = pool.tile([128, K * T * BL], dt_in, tag="xr")
            nc.sync.dma_start(xr_t[:], xr[:, :])

            wt_tiles = []
            for i in range(NW):
                wtt = pool.tile([128, WCH], dt_in, tag=f"wt{i}")
                wt_tiles.append(wtt)
            nc.scalar.dma_start(wt_tiles[0][:], wt[:, 0 * WCH:1 * WCH])
            nc.sync.dma_start(wt_tiles[1][:], wt[:, 1 * WCH:2 * WCH])
            nc.gpsimd.dma_start(wt_tiles[2][:], wt[:, 2 * WCH:3 * WCH])
            nc.scalar.dma_start(wt_tiles[3][:], wt[:, 3 * WCH:4 * WCH])

            TB = T * BL  # 256
            t1 = pool.tile([128, TB], dt_in, tag="t1")
            t2 = pool.tile([128, TB], dt_in, tag="t2")
            y = pool.tile([128, TB], dt_in, tag="y")
            nc.vector.tensor_add(t1[:], xr_t[:, 0:TB], xr_t[:, TB:2 * TB])
            nc.vector.tensor_add(t2[:], xr_t[:, 2 * TB:3 * TB], xr_t[:, 3 * TB:4 * TB])
            nc.vector.tensor_add(y[:], t1[:], t2[:])

            ps = psum_pool.tile([BL, BL * OUT], mybir.dt.float32, tag="ps")
            for t in range(T):
                ci, lt = divmod(t, TCH)
                nc.tensor.matmul(
                    ps[:],
                    y[:, t * BL:(t + 1) * BL],
                    wt_tiles[ci][:, lt * BL * OUT:(lt + 1) * BL * OUT],
                    start=(t == 0),
                    stop=(t == T - 1),
                )

            out_sb = pool.tile([BL, BL * OUT], mybir.dt.float32, tag="out")
            nc.vector.tensor_copy(out_sb[:], ps[:])
            nc.sync.dma_start(out[:, :], out_sb[:])

    nc.compile()
    return nc


def _get_nc(dtype_name: str):
    if dtype_name not in _CACHE:
        _CACHE[dtype_name] = _build_nc(dtype_name)
    return _CACHE[dtype_name]


def _host_layouts(x: np.ndarray, w: np.ndarray, np_dt) -> list:
    """Build per-core input maps (pure layout permutation of the full inputs)."""
    x4 = x.reshape(B, H, Wd, C, K)
    # T_k[b] = rot90(x_b[..., k], -k): the k-th rotation-gathered copy of x
    TK = np.stack(
        [np.rot90(x4[..., k], -k, axes=(1, 2)) for k in range(K)], axis=1
    )  # (B, K, 8, 8, C)
    TKf = TK.reshape(B, K, T, 2, C)                      # [b, k, t, u, c]
    xr_all = TKf.transpose(3, 4, 1, 2, 0).reshape(128, K, T, B)

    wv = w.reshape(B, T, 128, OUT)                       # [b, t, p, o]
    wt_all = wv.transpose(2, 1, 0, 3)                    # [p, t, b, o]

    in_maps = []
    for m in range(NCORES):
        sl = slice(m * BL, (m + 1) * BL)
        xr_m = np.ascontiguousarray(xr_all[:, :, :, sl]).reshape(128, K * T * BL)
        wt_m = np.ascontiguousarray(wt_all[:, :, sl, :]).reshape(128, T * BL * OUT)
        in_maps.append({"xr": xr_m.astype(np_dt), "wt": wt_m.astype(np_dt)})
    return in_maps


last_results = None  # BassKernelResults of the most recent run (for test.py)


def kernel(inputs: np.ndarray, w: np.ndarray) -> np.ndarray:
    import ml_dtypes
    from concourse import bass_utils

    global last_results
    x = np.ascontiguousarray(np.asarray(inputs, dtype=np.float32))
    wf = np.ascontiguousarray(np.asarray(w, dtype=np.float32))
    np_dt = ml_dtypes.bfloat16 if DTYPE == "bf16" else np.float32

    in_maps = _host_layouts(x, wf, np_dt)
    nc = _get_nc(DTYPE)
    res = bass_utils.run_bass_kernel_spmd(nc, in_maps, core_ids=list(range(NCORES)))
    last_results = res
    # r["out"] is (8, 120); sample bl's outputs are the diagonal block
    out = np.stack(
        [
            r["out"][bl, bl * OUT:(bl + 1) * OUT]
            for r in res.results
            for bl in range(BL)
        ],
        axis=0,
    )
    return out.reshape(B, OUT, 1).astype(np.float32)
